# revision 18
# baseline (speedup 1.0000x reference)
"""Trainium2 Bass kernel: autoregressive GRU decoder (nn_Decoder).

B=1024, T=128, H=1024, I=128 (POSE=96 + TRAJ=32).
Data-parallel over batch across 8 NeuronCores (128 rows/core), no collectives.

Layout: fully transposed on-device — features on partitions, batch on the
free dim. h state kept as 4 k-pair tiles [128, 2, 128]; x state [128, 128].
Matmul operands bf16, state fp32, PSUM accumulation fp32.

The pose/fc output head is folded into a single matmul:
tp = [[fc_p@lp_W + fc_h], [lp_W]] @ h' + btp, so y = x + tp in one shot.

Emission is k-major (default, _build_kmaj): for each h k-tile g, all 24
gate m-accumulators take their k=g contribution together, so step t+1's
matmul stream starts as soon as chain t finishes its first h' pair instead
of waiting for the full elementwise chain (HW: 29.7 -> ~15 us/step).
PSUM banks hold 4 m-slices as ONE accumulation group each (2KB zero-region
rule); per-bank tags (bufs=1) pair each bank's next-step start-matmul with
early chain readers of step t.  The chain computes the bf16 h' (which gates
the PE) before the f32 state copy, keeps everything on DVE+ACT (GpSimd is
far slower on HW), and runs chunk 0 at single-k granularity to shorten the
per-step structural latency.
"""

import os
import sys

if "/opt/trn_rl_repo" not in sys.path:
    sys.path.insert(0, "/opt/trn_rl_repo")

import numpy as np
import ml_dtypes

B, T, H = 1024, 128, 1024
POSE, TRAJ = 96, 32
I = POSE + TRAJ  # 128
NCORES = 8
BL = B // NCORES  # 128 batch rows per core
KH = H // 128  # 8 h K-tiles
P = 128

# chunks (in units of 128-wide k-tiles) for the elementwise gate pipeline
_SC = [(0, 4), (4, 6), (6, 7), (7, 8)]
_CHUNK_OF = [0] * 4 + [1] * 2 + [2] + [3]

_BUILD_CACHE = {}
LAST_RESULTS = None


def _build_mmaj(t_steps, reps=1, skeleton=False, pool_chain=False, bufs=2):
    """Legacy m-major emission. skeleton=True emits only the matmul stream
    (timing experiments). reps>1 wraps the step loop in For_i (skeleton
    only; the full chain deadlocks the scheduler under For_i).
    pool_chain: run d/e/h'/cast on GpSimd (False -> DVE; HW-measured
    1.4us/step faster on DVE despite sim preferring GpSimd)."""
    import contextlib

    import concourse.bass as bass
    import concourse.tile as tile
    from concourse import bacc, mybir

    f32 = mybir.dt.float32
    bf16 = mybir.dt.bfloat16
    AF = mybir.ActivationFunctionType
    OP = mybir.AluOpType

    nc = bacc.Bacc(None, target_bir_lowering=False, debug=False)

    # ---- DRAM I/O ------------------------------------------------------
    dp = nc.declare_dram_parameter
    x0_d = dp("x0", [P, BL], f32, isOutput=False)             # x0^T
    h0_d = dp("h0", [P, KH, BL], f32, isOutput=False)         # h0^T k-tiles
    wrz_d = dp("wrz", [P, 9, 16, P], bf16, isOutput=False)    # [p,k,m,j] k0=x
    wnx_d = dp("wnx", [P, KH, P], bf16, isOutput=False)       # Win^T
    wnh_d = dp("wnh", [P, KH, KH, P], bf16, isOutput=False)   # Whn^T [p,k,m,j]
    wtp_d = dp("wtp", [P, KH, P], bf16, isOutput=False)       # tp weights^T
    brz_d = dp("brz", [P, 16], f32, isOutput=False)           # col m = bias m-tile
    bxn_d = dp("bxn", [P, KH], f32, isOutput=False)
    bhn_d = dp("bhn", [P, KH], f32, isOutput=False)
    btp_d = dp("btp", [P, 1], f32, isOutput=False)            # [lp_b; fc_b]
    yt_d = dp("yt", [t_steps, P, BL], f32, isOutput=True)     # y^T per step

    with tile.TileContext(nc) as tc:
        with (
            tc.tile_pool(name="const", bufs=1) as cpool,
            tc.tile_pool(name="state", bufs=bufs) as spool,
            tc.tile_pool(name="work", bufs=bufs) as wpool,
            tc.tile_pool(name="gates_ps", bufs=7, space="PSUM") as gpool,
            tc.tile_pool(name="tp_ps", bufs=1, space="PSUM") as tpool,
        ):
            # ---- one-time loads ----------------------------------------
            def load_const(dram, shape, dtype):
                t = cpool.tile(shape, dtype, tag=dram.name)
                nc.sync.dma_start(t[:], dram[:])
                return t

            wrz_s = load_const(wrz_d, [P, 9, 16, P], bf16)
            wnx_s = load_const(wnx_d, [P, KH, P], bf16)
            wnh_s = load_const(wnh_d, [P, KH, KH, P], bf16)
            wtp_s = load_const(wtp_d, [P, KH, P], bf16)
            brz_s = load_const(brz_d, [P, 16], f32)
            bxn_s = load_const(bxn_d, [P, KH], f32)
            bhn_s = load_const(bhn_d, [P, KH], f32)
            btp_s = load_const(btp_d, [P, 1], f32)

            h_f = [
                spool.tile([P, c1 - c0, BL], f32, tag=f"hf{i}", name=f"hf{i}")
                for i, (c0, c1) in enumerate(_SC)
            ]
            h_b = [
                spool.tile([P, c1 - c0, BL], bf16, tag=f"hb{i}", name=f"hb{i}")
                for i, (c0, c1) in enumerate(_SC)
            ]
            for i, (c0, c1) in enumerate(_SC):
                nc.sync.dma_start(h_f[i][:], h0_d[:, c0:c1, :])
                nc.vector.tensor_copy(h_b[i][:], h_f[i][:])
            x_f = spool.tile([P, BL], f32, tag="xf")
            nc.sync.dma_start(x_f[:], x0_d[:])
            x_b = spool.tile([P, BL], bf16, tag="xb")
            nc.vector.tensor_copy(x_b[:], x_f[:])

            def hbk(k):  # bf16 h k-tile accessor (chunked state tiles)
                i = _CHUNK_OF[k]
                return h_b[i][:, k - _SC[i][0], :]

            # ---- time steps --------------------------------------------
            HM = KH // 2  # m-tiles per 1-bank psum tile

            rep_ctx = (
                tc.For_i(0, reps, 1) if reps > 1 else contextlib.nullcontext()
            )
            with rep_ctx:
             for t in range(t_steps):
                 # One PSUM bank per tile ([128, 4, 128] fp32) so banks free
                 # individually.  m-tile m lives in (pair, m % 4).
                 ps_r = [
                     gpool.tile([P, 2, BL], f32, tag="ps", name=f"psr{i}_{t}")
                     for i in range(4)
                 ]
                 ps_hn = [
                     gpool.tile([P, 2, BL], f32, tag="ps", name=f"pshn{i}_{t}")
                     for i in range(4)
                 ]
                 ps_xn = [
                     gpool.tile([P, HM, BL], f32, tag="ps", name=f"psxn{i}_{t}")
                     for i in range(2)
                 ]
                 # z in 2-m-tile tiles: the tail sigmoids wait only on their
                 # own bank's matmuls instead of all of z.
                 _ZB = [(0, 2), (2, 4), (4, 6), (6, 7), (7, 8)]
                 ps_z = [
                     gpool.tile([P, z1 - z0, BL], f32, tag="ps",
                                name=f"psz{i}_{t}")
                     for i, (z0, z1) in enumerate(_ZB)
                 ]

                 def sl(pair, m):
                     return pair[m // HM][:, m % HM, :]

                 def slz(m):
                     for i, (z0, z1) in enumerate(_ZB):
                         if z0 <= m < z1:
                             return ps_z[i][:, m - z0, :]

                 def mm_r(m):
                     out = ps_r[m // 2][:, m % 2, :]
                     for k in range(KH):
                         nc.tensor.matmul(
                             out, wrz_s[:, 1 + k, m, :], hbk(k),
                             start=(k == 0), stop=False,
                         )
                     nc.tensor.matmul(
                         out, wrz_s[:, 0, m, :], x_b[:], start=False, stop=True
                     )

                 def mm_hn(m):
                     out = ps_hn[m // 2][:, m % 2, :]
                     for k in range(KH):
                         nc.tensor.matmul(
                             out, wnh_s[:, k, m, :], hbk(k),
                             start=(k == 0), stop=(k == KH - 1),
                         )

                 # PE emission order: r/hn pairs (chain-critical first), xn
                 # early (needs only x), z last (shallow post-chain).
                 mm_r(0); mm_hn(0); mm_r(1); mm_hn(1)
                 for m in range(KH):
                     nc.tensor.matmul(
                         sl(ps_xn, m), wnx_s[:, m, :], x_b[:],
                         start=True, stop=True,
                     )
                 for m in range(2, KH):
                     mm_r(m); mm_hn(m)
                 for m in range(KH):
                     out = slz(m)
                     for k in range(KH):
                         nc.tensor.matmul(
                             out, wrz_s[:, 1 + k, KH + m, :], hbk(k),
                             start=(k == 0), stop=False,
                         )
                     nc.tensor.matmul(
                         out, wrz_s[:, 0, KH + m, :], x_b[:],
                         start=False, stop=True,
                     )

                 if skeleton:
                     continue  # timing experiment: matmul stream only

                 # Chunked per-tile pipeline: every chunk tensor is its own
                 # tile so readers wait only on their chunk's writers.
                 r_s = [
                     wpool.tile([P, 2, BL], f32, tag=f"r{i}", name=f"r{i}_{t}")
                     for i in range(4)
                 ]
                 t1 = [
                     wpool.tile([P, 2, BL], f32, tag=f"t1{i}", name=f"t1{i}_{t}")
                     for i in range(4)
                 ]
                 t2c = [
                     wpool.tile([P, c1 - c0, BL], f32, tag=f"t2{i}",
                                name=f"t2{i}_{t}")
                     for i, (c0, c1) in enumerate(_SC)
                 ]
                 n_c = [
                     wpool.tile([P, c1 - c0, BL], f32, tag=f"n{i}",
                                name=f"n{i}_{t}")
                     for i, (c0, c1) in enumerate(_SC)
                 ]
                 d_c = [
                     wpool.tile([P, c1 - c0, BL], f32, tag=f"d{i}",
                                name=f"d{i}_{t}")
                     for i, (c0, c1) in enumerate(_SC)
                 ]
                 z_c = [
                     wpool.tile([P, c1 - c0, BL], f32, tag=f"z{i}",
                                name=f"z{i}_{t}")
                     for i, (c0, c1) in enumerate(_SC)
                 ]
                 e_c = [
                     wpool.tile([P, c1 - c0, BL], f32, tag=f"e{i}",
                                name=f"e{i}_{t}")
                     for i, (c0, c1) in enumerate(_SC)
                 ]
                 hf2 = [
                     spool.tile([P, c1 - c0, BL], f32, tag=f"hf{i}",
                                name=f"hf{i}_{t}")
                     for i, (c0, c1) in enumerate(_SC)
                 ]
                 hb2 = [
                     spool.tile([P, c1 - c0, BL], bf16, tag=f"hb{i}",
                                name=f"hb{i}_{t}")
                     for i, (c0, c1) in enumerate(_SC)
                 ]

                 def t2sl(m):
                     i = _CHUNK_OF[m]
                     return t2c[i][:, m - _SC[i][0], :]

                 def zsl(m):
                     i = _CHUNK_OF[m]
                     return z_c[i][:, m - _SC[i][0], :]

                 def sig_r(m):
                     nc.scalar.activation(
                         r_s[m // 2][:, m % 2, :], ps_r[m // 2][:, m % 2, :],
                         AF.Sigmoid, bias=brz_s[:, m : m + 1],
                     )

                 def t12(m):
                     nc.vector.scalar_tensor_tensor(
                         t1[m // 2][:, m % 2, :], ps_hn[m // 2][:, m % 2, :],
                         bhn_s[:, m : m + 1], r_s[m // 2][:, m % 2, :],
                         op0=OP.add, op1=OP.mult,
                     )
                     nc.vector.scalar_tensor_tensor(
                         t2sl(m), sl(ps_xn, m), bxn_s[:, m : m + 1],
                         t1[m // 2][:, m % 2, :], op0=OP.add, op1=OP.add,
                     )

                 def tanh_chunk(i):
                     nc.scalar.activation(n_c[i][:], t2c[i][:], AF.Tanh)

                 chain = nc.gpsimd if pool_chain else nc.vector

                 def d_chunk(i):
                     chain.tensor_sub(d_c[i][:], h_f[i][:], n_c[i][:])

                 def sig_z(m):
                     nc.scalar.activation(
                         zsl(m), slz(m), AF.Sigmoid,
                         bias=brz_s[:, KH + m : KH + m + 1],
                     )

                 def ehc_chunk(i, eng=None):
                     eng = eng or chain
                     eng.tensor_mul(e_c[i][:], z_c[i][:], d_c[i][:])
                     eng.tensor_add(hf2[i][:], n_c[i][:], e_c[i][:])
                     eng.tensor_copy(hb2[i][:], hf2[i][:])

                 # Emission interleave: per-engine order matches readiness
                 sig_r(0); sig_r(1); sig_r(2); sig_r(3)
                 t12(0); t12(1); t12(2); t12(3)
                 sig_r(4); sig_r(5)
                 t12(4); t12(5)
                 tanh_chunk(0)
                 sig_r(6); sig_r(7)
                 t12(6); t12(7)
                 tanh_chunk(1)
                 for m in range(4):
                     sig_z(m)
                 tanh_chunk(2); tanh_chunk(3)
                 for m in range(4, KH):
                     sig_z(m)

                 d_chunk(0); d_chunk(1)
                 ehc_chunk(0, nc.vector)
                 d_chunk(2); d_chunk(3)
                 ehc_chunk(1); ehc_chunk(2); ehc_chunk(3)

                 # tp = [[lp_W],[fc_p@lp_W + fc_h]] @ h_n  (one matmul set)
                 ps_tp_t = tpool.tile(
                     [P, HM, BL], f32, tag="tp", name=f"pstp_{t}"
                 )
                 ps_tp = ps_tp_t[:, 0, :]
                 for k in range(KH):
                     i = _CHUNK_OF[k]
                     nc.tensor.matmul(
                         ps_tp, wtp_s[:, k, :], hb2[i][:, k - _SC[i][0], :],
                         start=(k == 0), stop=(k == KH - 1),
                     )

                 # y = x + tp + btp ; y becomes x
                 x_f2 = spool.tile([P, BL], f32, tag="xf")
                 nc.vector.scalar_tensor_tensor(
                     x_f2[:], ps_tp, btp_s[:, 0:1], x_f[:],
                     op0=OP.add, op1=OP.add,
                 )
                 x_b2 = spool.tile([P, BL], bf16, tag="xb")
                 nc.vector.tensor_copy(x_b2[:], x_f2[:])
                 nc.sync.dma_start(yt_d[t, :, :], x_f2[:])

                 x_f, x_b, h_f, h_b = x_f2, x_b2, hf2, hb2

    nc.compile()
    return nc


def _build_kmaj(
    t_steps, reps=1, skeleton=False, gps_tail=False,
    hf2_gps=False, c0_singles=True, sr_first=True,
):
    """k-major emission: gate matmuls grouped by h k-tile so the PE starts
    step t+1's stream as soon as chain t finishes its first h' pair, instead
    of waiting for the full chain.

    PSUM: per-accumulator tags (bufs=1) so a start-matmul of step t+1 waits
    only on the *early* chain ops of step t that read the same slot:
      r(m) slot <- sigmoid_r(t,m); hn(m) <- t1(t,m); z(m) <- sigmoid_z(t,m);
      xn pair <- t2(t,m..); tp rides xn0's slot (its next-step user already
      depends on y(t) via x).
    z accumulators start at k-group 1 (k=0 made up during the x-pass) so
    their start-matmuls arrive after sigmoid_z(t,m) has freed the slot.

    reps>1 wraps the step loop in For_i with state copy-back (timing).
    """
    import contextlib

    import concourse.bass as bass  # noqa: F401
    import concourse.tile as tile
    from concourse import bacc, mybir

    f32 = mybir.dt.float32
    bf16 = mybir.dt.bfloat16
    AF = mybir.ActivationFunctionType
    OP = mybir.AluOpType

    nc = bacc.Bacc(None, target_bir_lowering=False, debug=False)

    dp = nc.declare_dram_parameter
    x0_d = dp("x0", [P, BL], f32, isOutput=False)
    h0_d = dp("h0", [P, KH, BL], f32, isOutput=False)
    wrz_d = dp("wrz", [P, 9, 16, P], bf16, isOutput=False)
    wnx_d = dp("wnx", [P, KH, P], bf16, isOutput=False)
    wnh_d = dp("wnh", [P, KH, KH, P], bf16, isOutput=False)
    wtp_d = dp("wtp", [P, KH, P], bf16, isOutput=False)
    brz_d = dp("brz", [P, 16], f32, isOutput=False)
    bxn_d = dp("bxn", [P, KH], f32, isOutput=False)
    bhn_d = dp("bhn", [P, KH], f32, isOutput=False)
    btp_d = dp("btp", [P, 1], f32, isOutput=False)
    yt_d = dp("yt", [t_steps, P, BL], f32, isOutput=True)

    NP = KH // 2  # h-state pairs

    with tile.TileContext(nc) as tc:
        with (
            tc.tile_pool(name="const", bufs=1) as cpool,
            tc.tile_pool(name="state", bufs=2) as spool,
            tc.tile_pool(name="work", bufs=2) as wpool,
            tc.tile_pool(name="gates_ps", bufs=1, space="PSUM") as gp,
        ):
            def load_const(dram, shape, dtype):
                t = cpool.tile(shape, dtype, tag=dram.name, name=dram.name)
                nc.sync.dma_start(t[:], dram[:])
                return t

            wrz_s = load_const(wrz_d, [P, 9, 16, P], bf16)
            wnx_s = load_const(wnx_d, [P, KH, P], bf16)
            wnh_s = load_const(wnh_d, [P, KH, KH, P], bf16)
            wtp_s = load_const(wtp_d, [P, KH, P], bf16)
            brz_s = load_const(brz_d, [P, 16], f32)
            bxn_s = load_const(bxn_d, [P, KH], f32)
            bhn_s = load_const(bhn_d, [P, KH], f32)
            btp_s = load_const(btp_d, [P, 1], f32)

            # initial state: cpool (bufs=1) so reps>1 can copy back into it
            hin_f = [
                cpool.tile([P, 2, BL], f32, tag=f"hinf{i}", name=f"hinf{i}")
                for i in range(NP)
            ]
            hin_b = [
                cpool.tile([P, 2, BL], bf16, tag=f"hinb{i}", name=f"hinb{i}")
                for i in range(NP)
            ]
            for i in range(NP):
                nc.sync.dma_start(hin_f[i][:], h0_d[:, 2 * i : 2 * i + 2, :])
                nc.vector.tensor_copy(hin_b[i][:], hin_f[i][:])
            xin_f = cpool.tile([P, BL], f32, tag="xinf", name="xinf")
            nc.sync.dma_start(xin_f[:], x0_d[:])
            xin_b = cpool.tile([P, BL], bf16, tag="xinb", name="xinb")
            nc.vector.tensor_copy(xin_b[:], xin_f[:])

            T12E = nc.vector  # t1/t2 read PSUM; GpSimd has no PSUM port
            TAILE = nc.gpsimd if gps_tail else nc.vector

            rep_ctx = (
                tc.For_i(0, reps, 1) if reps > 1 else contextlib.nullcontext()
            )
            with rep_ctx:
                x_f, x_b = xin_f, xin_b
                h_f, h_b = list(hin_f), list(hin_b)

                for t in range(t_steps):
                    def hbk(k):
                        return h_b[k // 2][:, k % 2, :]

                    # PSUM: 8 banks, one [P,4,BL] tile each (bank-granular
                    # slots); 4 m-slices per bank, subtile deps keep reader
                    # granularity fine.
                    ps_r = [
                        gp.tile([P, 4, BL], f32, tag=f"r{i}", name=f"psr{i}_{t}")
                        for i in range(2)
                    ]
                    ps_hn = [
                        gp.tile([P, 4, BL], f32, tag=f"hn{i}", name=f"pshn{i}_{t}")
                        for i in range(2)
                    ]
                    ps_z = [
                        gp.tile([P, 4, BL], f32, tag=f"z{i}", name=f"psz{i}_{t}")
                        for i in range(2)
                    ]
                    ps_xn = [
                        gp.tile([P, 4, BL], f32, tag=f"xn{i}", name=f"psxn{i}_{t}")
                        for i in range(2)
                    ]

                    def rsl(m):
                        return ps_r[m // 4][:, m % 4, :]

                    def hnsl(m):
                        return ps_hn[m // 4][:, m % 4, :]

                    def zsl(m):
                        return ps_z[m // 4][:, m % 4, :]

                    def xnsl(m):
                        return ps_xn[m // 4][:, m % 4, :]

                    # ---- gate matmul stream, k-major ----------------------
                    # One accumulation group per PSUM bank (2KB zero region):
                    # start only on the first matmul touching the bank, stop
                    # only on the last; fresh slices are overwritten via
                    # per-element has_written bits.
                    for g in range(KH):
                        for m in range(KH):
                            nc.tensor.matmul(
                                rsl(m), wrz_s[:, 1 + g, m, :], hbk(g),
                                start=(g == 0 and m % 4 == 0), stop=False,
                            )
                            nc.tensor.matmul(
                                hnsl(m), wnh_s[:, g, m, :], hbk(g),
                                start=(g == 0 and m % 4 == 0),
                                stop=(g == KH - 1 and m % 4 == 3),
                            )
                        if g >= 1:  # z starts at k-group 1 (k=0 made up below)
                            for m in range(KH):
                                nc.tensor.matmul(
                                    zsl(m), wrz_s[:, 1 + g, KH + m, :], hbk(g),
                                    start=(g == 1 and m % 4 == 0), stop=False,
                                )
                    # x-pass (+ z k=0 makeup), m-interleaved
                    for m in range(KH):
                        nc.tensor.matmul(
                            zsl(m), wrz_s[:, 1, KH + m, :], hbk(0),
                            start=False, stop=False,
                        )
                        nc.tensor.matmul(
                            rsl(m), wrz_s[:, 0, m, :], x_b[:],
                            start=False, stop=(m % 4 == 3),
                        )
                        nc.tensor.matmul(
                            zsl(m), wrz_s[:, 0, KH + m, :], x_b[:],
                            start=False, stop=(m % 4 == 3),
                        )
                        nc.tensor.matmul(
                            xnsl(m), wnx_s[:, m, :], x_b[:],
                            start=(m % 4 == 0), stop=(m % 4 == 3),
                        )

                    if skeleton:
                        continue

                    # ---- chain ------------------------------------------
                    r_s = [
                        wpool.tile([P, 2, BL], f32, tag=f"rs{i}", name=f"rs{i}_{t}")
                        for i in range(4)
                    ]
                    z_s = [
                        wpool.tile([P, 2, BL], f32, tag=f"zs{i}", name=f"zs{i}_{t}")
                        for i in range(4)
                    ]
                    t1 = [
                        wpool.tile([P, BL], f32, tag=f"t1{m}", name=f"t1{m}_{t}")
                        for m in range(KH)
                    ]
                    t2 = [
                        wpool.tile([P, 2, BL], f32, tag=f"t2{i}", name=f"t2{i}_{t}")
                        for i in range(4)
                    ]
                    n_s = [
                        wpool.tile([P, 2, BL], f32, tag=f"n{i}", name=f"n{i}_{t}")
                        for i in range(4)
                    ]
                    d_c = [
                        wpool.tile([P, 2, BL], f32, tag=f"d{i}", name=f"d{i}_{t}")
                        for i in range(4)
                    ]
                    e_c = [
                        wpool.tile([P, 2, BL], f32, tag=f"e{i}", name=f"e{i}_{t}")
                        for i in range(4)
                    ]
                    hf2 = [
                        spool.tile([P, 2, BL], f32, tag=f"hf{i}", name=f"hf{i}_{t}")
                        for i in range(NP)
                    ]
                    hb2 = [
                        spool.tile([P, 2, BL], bf16, tag=f"hb{i}", name=f"hb{i}_{t}")
                        for i in range(NP)
                    ]
                    ps_tp = gp.tile([P, 1, BL], f32, tag="xn0", name=f"pstp_{t}")

                    def sig_r(m):
                        nc.scalar.activation(
                            r_s[m // 2][:, m % 2, :], rsl(m), AF.Sigmoid,
                            bias=brz_s[:, m : m + 1],
                        )

                    def sig_z(m):
                        nc.scalar.activation(
                            z_s[m // 2][:, m % 2, :], zsl(m), AF.Sigmoid,
                            bias=brz_s[:, KH + m : KH + m + 1],
                        )

                    def t12(m):
                        T12E.scalar_tensor_tensor(
                            t1[m][:], hnsl(m), bhn_s[:, m : m + 1],
                            r_s[m // 2][:, m % 2, :], op0=OP.add, op1=OP.mult,
                        )
                        T12E.scalar_tensor_tensor(
                            t2[m // 2][:, m % 2, :], xnsl(m),
                            bxn_s[:, m : m + 1], t1[m][:],
                            op0=OP.add, op1=OP.add,
                        )

                    def tanh_p(i):
                        nc.scalar.activation(n_s[i][:], t2[i][:], AF.Tanh)

                    HF2E = nc.gpsimd if hf2_gps else TAILE

                    def tail_p(i):
                        # h' = n + z*(h - n); bf16 form first (gates the PE),
                        # f32 state copy off the critical path
                        TAILE.tensor_sub(d_c[i][:], h_f[i][:], n_s[i][:])
                        TAILE.tensor_mul(e_c[i][:], z_s[i][:], d_c[i][:])
                        TAILE.tensor_add(hb2[i][:], n_s[i][:], e_c[i][:])
                        HF2E.tensor_add(hf2[i][:], n_s[i][:], e_c[i][:])

                    def tail_s(i, j):
                        # single-k tail (lower latency for chunk 0)
                        sl = (slice(None), slice(j, j + 1), slice(None))
                        nc.scalar.activation(n_s[i][sl], t2[i][sl], AF.Tanh)
                        TAILE.tensor_sub(d_c[i][sl], h_f[i][sl], n_s[i][sl])
                        TAILE.tensor_mul(e_c[i][sl], z_s[i][sl], d_c[i][sl])
                        TAILE.tensor_add(hb2[i][sl], n_s[i][sl], e_c[i][sl])
                        HF2E.tensor_add(hf2[i][sl], n_s[i][sl], e_c[i][sl])

                    def tp_mm(k):
                        nc.tensor.matmul(
                            ps_tp[:, 0, :], wtp_s[:, k, :],
                            hb2[k // 2][:, k % 2, :],
                            start=(k == 0), stop=(k == KH - 1),
                        )

                    def emit_pair(i):
                        if c0_singles and i == 0:
                            tail_s(0, 0)
                            tp_mm(0)
                            tail_s(0, 1)
                            tp_mm(1)
                            return
                        tanh_p(i)
                        tail_p(i)
                        tp_mm(2 * i)
                        tp_mm(2 * i + 1)

                    if sr_first:
                        # ACT order: all sigmoid_r first (critical: t1 needs
                        # r), sigmoid_z just before its consumers
                        for m in range(KH):
                            sig_r(m)
                            t12(m)
                            if m >= 2 and m % 2 == 0:
                                i = m // 2 - 1
                                sig_z(2 * i)
                                sig_z(2 * i + 1)
                                emit_pair(i)
                        sig_z(6)
                        sig_z(7)
                        emit_pair(3)
                    else:
                        for m in range(KH):
                            sig_r(m)
                            sig_z(m)
                            t12(m)
                            if m >= 2 and m % 2 == 0:
                                emit_pair(m // 2 - 1)
                        emit_pair(3)

                    # y = x + tp + btp ; y becomes x (bf16 first: gates x-pass)
                    x_b2 = spool.tile([P, BL], bf16, tag="xb", name=f"xb_{t}")
                    nc.vector.scalar_tensor_tensor(
                        x_b2[:], ps_tp[:, 0, :], btp_s[:, 0:1], x_f[:],
                        op0=OP.add, op1=OP.add,
                    )
                    x_f2 = spool.tile([P, BL], f32, tag="xf", name=f"xf_{t}")
                    nc.vector.scalar_tensor_tensor(
                        x_f2[:], ps_tp[:, 0, :], btp_s[:, 0:1], x_f[:],
                        op0=OP.add, op1=OP.add,
                    )
                    nc.sync.dma_start(yt_d[t, :, :], x_f2[:])

                    x_f, x_b, h_f, h_b = x_f2, x_b2, hf2, hb2

                if reps > 1 and not skeleton:
                    for i in range(NP):
                        nc.vector.tensor_copy(hin_f[i][:], h_f[i][:])
                        nc.vector.tensor_copy(hin_b[i][:], h_b[i][:])
                    nc.vector.tensor_copy(xin_f[:], x_f[:])
                    nc.vector.tensor_copy(xin_b[:], x_b[:])

    nc.compile()
    return nc


def _build(t_steps, reps=1, skeleton=False, order=None, **kw):
    if order is None:
        order = os.environ.get("KERNEL_ORDER", "kmaj")
    if order == "kmaj":
        return _build_kmaj(t_steps, reps=reps, skeleton=skeleton, **kw)
    return _build_mmaj(t_steps, reps=reps, skeleton=skeleton, **kw)


def _prep_inputs(h, gt, Wih, Whh, bih, bhh, lp_W, lp_b, fc_W, fc_b):
    """Host-side: transpose into kernel layouts, cast weights to bf16."""
    bf = ml_dtypes.bfloat16
    f32 = np.float32

    # rz combined weights, transposed: [1152, 2048] -> [p, k(9), m(16), j]
    wrzT = np.concatenate([Wih[: 2 * H].T, Whh[: 2 * H].T], axis=0)
    wrz = np.empty((P, 9, 16, P), dtype=bf)
    for k in range(9):
        for m in range(16):
            wrz[:, k, m, :] = wrzT[k * P : (k + 1) * P, m * P : (m + 1) * P]

    wnxT = Wih[2 * H :].T  # [128, 1024]
    wnx = np.ascontiguousarray(wnxT.reshape(P, KH, P), dtype=bf)  # [p, m, j]

    wnhT = Whh[2 * H :].T  # [1024, 1024]
    wnh = np.empty((P, KH, KH, P), dtype=bf)
    for k in range(KH):
        for m in range(KH):
            wnh[:, k, m, :] = wnhT[k * P : (k + 1) * P, m * P : (m + 1) * P]

    # fold pose->traj head: traj = (fc_p@lp_W + fc_h)@h + (fc_p@lp_b + fc_b)
    fc_p = fc_W[:, :POSE].astype(np.float64)
    fc_h = fc_W[:, POSE:].astype(np.float64)
    m_traj = fc_p @ lp_W.astype(np.float64) + fc_h          # [32, 1024]
    m_tp = np.concatenate([m_traj, lp_W.astype(np.float64)], axis=0)  # [I, H]
    b_traj = fc_p @ lp_b.astype(np.float64) + fc_b          # [32]
    b_tp = np.concatenate([b_traj, lp_b.astype(np.float64)])  # [I]
    wtpT = m_tp.T  # [1024, 128]
    wtp = np.ascontiguousarray(
        wtpT.reshape(KH, P, P).transpose(1, 0, 2), dtype=bf
    )  # [p, k, m]

    b_rz = (bih + bhh)[: 2 * H].astype(f32)  # [2048]
    brz = np.ascontiguousarray(b_rz.reshape(16, P).T)  # [128, 16]
    bxn = np.ascontiguousarray(bih[2 * H :].reshape(KH, P).T.astype(f32))
    bhn = np.ascontiguousarray(bhh[2 * H :].reshape(KH, P).T.astype(f32))
    btp = b_tp.reshape(P, 1).astype(f32)

    shared = {
        "wrz": wrz, "wnx": wnx, "wnh": wnh, "wtp": wtp,
        "brz": brz, "bxn": bxn, "bhn": bhn, "btp": btp,
    }

    in_maps = []
    for c in range(NCORES):
        sl = slice(c * BL, (c + 1) * BL)
        x0 = np.ascontiguousarray(gt[sl, 0, :].T.astype(f32))  # [I, BL]
        h0 = np.ascontiguousarray(
            h[sl, :].T.reshape(KH, P, BL).transpose(1, 0, 2).astype(f32)
        )  # [p, k, b] = h[b, k*128+p]
        in_maps.append({"x0": x0, "h0": h0, **shared})
    return in_maps


def kernel(h, gt, Wih, Whh, bih, bhh, lp_W, lp_b, fc_W, fc_b, time_steps):
    from concourse.bass_utils import run_bass_kernel_spmd

    t_steps = int(time_steps)

    h = np.asarray(h, np.float32)
    gt = np.asarray(gt, np.float32)

    if t_steps not in _BUILD_CACHE:
        _BUILD_CACHE[t_steps] = _build(t_steps)
    nc = _BUILD_CACHE[t_steps]

    in_maps = _prep_inputs(
        h, gt, np.asarray(Wih, np.float32), np.asarray(Whh, np.float32),
        np.asarray(bih, np.float32), np.asarray(bhh, np.float32),
        np.asarray(lp_W, np.float32), np.asarray(lp_b, np.float32),
        np.asarray(fc_W, np.float32), np.asarray(fc_b, np.float32),
    )

    import os

    trace = bool(os.environ.get("KERNEL_TRACE"))
    res = run_bass_kernel_spmd(
        nc, in_maps, core_ids=list(range(NCORES)), trace=trace
    )
    global LAST_RESULTS
    LAST_RESULTS = res

    out = np.empty((B, t_steps, I), dtype=np.float32)
    for c in range(NCORES):
        yt = res.results[c]["yt"]  # [T, I_k, BL]
        out[c * BL : (c + 1) * BL] = yt.transpose(2, 0, 1)
    return out



# revision 22
# speedup vs baseline: 1.0505x; 1.0505x over previous
"""Trainium2 Bass kernel: autoregressive GRU decoder (nn_Decoder).

B=1024, T=128, H=1024, I=128 (POSE=96 + TRAJ=32).
Data-parallel over batch across 8 NeuronCores (128 rows/core), no collectives.

Layout: fully transposed on-device — features on partitions, batch on the
free dim. h state kept as 4 k-pair tiles [128, 2, 128]; x state [128, 128].
Matmul operands bf16, state fp32, PSUM accumulation fp32.

The pose/fc output head is folded into a single matmul:
tp = [[fc_p@lp_W + fc_h], [lp_W]] @ h' + btp, so y = x + tp in one shot.

Emission is k-major (default, _build_kmaj): for each h k-tile g, all 24
gate m-accumulators take their k=g contribution together, so step t+1's
matmul stream starts as soon as chain t finishes its first h' pair instead
of waiting for the full elementwise chain (HW: 29.7 -> ~15 us/step).
PSUM banks hold 4 m-slices as ONE accumulation group each (2KB zero-region
rule); per-bank tags (bufs=1) pair each bank's next-step start-matmul with
early chain readers of step t.  The chain computes the bf16 h' (which gates
the PE) before the f32 state copy, keeps everything on DVE+ACT (GpSimd is
far slower on HW), and runs chunk 0 at single-k granularity to shorten the
per-step structural latency.
"""

import os
import sys

if "/opt/trn_rl_repo" not in sys.path:
    sys.path.insert(0, "/opt/trn_rl_repo")

import numpy as np
import ml_dtypes

B, T, H = 1024, 128, 1024
POSE, TRAJ = 96, 32
I = POSE + TRAJ  # 128
NCORES = 8
BL = B // NCORES  # 128 batch rows per core
KH = H // 128  # 8 h K-tiles
P = 128

# chunks (in units of 128-wide k-tiles) for the elementwise gate pipeline
_SC = [(0, 4), (4, 6), (6, 7), (7, 8)]
_CHUNK_OF = [0] * 4 + [1] * 2 + [2] + [3]

_BUILD_CACHE = {}
LAST_RESULTS = None


def _build_mmaj(t_steps, reps=1, skeleton=False, pool_chain=False, bufs=2):
    """Legacy m-major emission. skeleton=True emits only the matmul stream
    (timing experiments). reps>1 wraps the step loop in For_i (skeleton
    only; the full chain deadlocks the scheduler under For_i).
    pool_chain: run d/e/h'/cast on GpSimd (False -> DVE; HW-measured
    1.4us/step faster on DVE despite sim preferring GpSimd)."""
    import contextlib

    import concourse.bass as bass
    import concourse.tile as tile
    from concourse import bacc, mybir

    f32 = mybir.dt.float32
    bf16 = mybir.dt.bfloat16
    AF = mybir.ActivationFunctionType
    OP = mybir.AluOpType

    nc = bacc.Bacc(None, target_bir_lowering=False, debug=False)

    # ---- DRAM I/O ------------------------------------------------------
    dp = nc.declare_dram_parameter
    x0_d = dp("x0", [P, BL], f32, isOutput=False)             # x0^T
    h0_d = dp("h0", [P, KH, BL], f32, isOutput=False)         # h0^T k-tiles
    wrz_d = dp("wrz", [P, 9, 16, P], bf16, isOutput=False)    # [p,k,m,j] k0=x
    wnx_d = dp("wnx", [P, KH, P], bf16, isOutput=False)       # Win^T
    wnh_d = dp("wnh", [P, KH, KH, P], bf16, isOutput=False)   # Whn^T [p,k,m,j]
    wtp_d = dp("wtp", [P, KH, P], bf16, isOutput=False)       # tp weights^T
    brz_d = dp("brz", [P, 16], f32, isOutput=False)           # col m = bias m-tile
    bxn_d = dp("bxn", [P, KH], f32, isOutput=False)
    bhn_d = dp("bhn", [P, KH], f32, isOutput=False)
    btp_d = dp("btp", [P, 1], f32, isOutput=False)            # [lp_b; fc_b]
    yt_d = dp("yt", [t_steps, P, BL], f32, isOutput=True)     # y^T per step

    with tile.TileContext(nc) as tc:
        with (
            tc.tile_pool(name="const", bufs=1) as cpool,
            tc.tile_pool(name="state", bufs=bufs) as spool,
            tc.tile_pool(name="work", bufs=bufs) as wpool,
            tc.tile_pool(name="gates_ps", bufs=7, space="PSUM") as gpool,
            tc.tile_pool(name="tp_ps", bufs=1, space="PSUM") as tpool,
        ):
            # ---- one-time loads ----------------------------------------
            def load_const(dram, shape, dtype):
                t = cpool.tile(shape, dtype, tag=dram.name)
                nc.sync.dma_start(t[:], dram[:])
                return t

            wrz_s = load_const(wrz_d, [P, 9, 16, P], bf16)
            wnx_s = load_const(wnx_d, [P, KH, P], bf16)
            wnh_s = load_const(wnh_d, [P, KH, KH, P], bf16)
            wtp_s = load_const(wtp_d, [P, KH, P], bf16)
            brz_s = load_const(brz_d, [P, 16], f32)
            bxn_s = load_const(bxn_d, [P, KH], f32)
            bhn_s = load_const(bhn_d, [P, KH], f32)
            btp_s = load_const(btp_d, [P, 1], f32)

            h_f = [
                spool.tile([P, c1 - c0, BL], f32, tag=f"hf{i}", name=f"hf{i}")
                for i, (c0, c1) in enumerate(_SC)
            ]
            h_b = [
                spool.tile([P, c1 - c0, BL], bf16, tag=f"hb{i}", name=f"hb{i}")
                for i, (c0, c1) in enumerate(_SC)
            ]
            for i, (c0, c1) in enumerate(_SC):
                nc.sync.dma_start(h_f[i][:], h0_d[:, c0:c1, :])
                nc.vector.tensor_copy(h_b[i][:], h_f[i][:])
            x_f = spool.tile([P, BL], f32, tag="xf")
            nc.sync.dma_start(x_f[:], x0_d[:])
            x_b = spool.tile([P, BL], bf16, tag="xb")
            nc.vector.tensor_copy(x_b[:], x_f[:])

            def hbk(k):  # bf16 h k-tile accessor (chunked state tiles)
                i = _CHUNK_OF[k]
                return h_b[i][:, k - _SC[i][0], :]

            # ---- time steps --------------------------------------------
            HM = KH // 2  # m-tiles per 1-bank psum tile

            rep_ctx = (
                tc.For_i(0, reps, 1) if reps > 1 else contextlib.nullcontext()
            )
            with rep_ctx:
             for t in range(t_steps):
                 # One PSUM bank per tile ([128, 4, 128] fp32) so banks free
                 # individually.  m-tile m lives in (pair, m % 4).
                 ps_r = [
                     gpool.tile([P, 2, BL], f32, tag="ps", name=f"psr{i}_{t}")
                     for i in range(4)
                 ]
                 ps_hn = [
                     gpool.tile([P, 2, BL], f32, tag="ps", name=f"pshn{i}_{t}")
                     for i in range(4)
                 ]
                 ps_xn = [
                     gpool.tile([P, HM, BL], f32, tag="ps", name=f"psxn{i}_{t}")
                     for i in range(2)
                 ]
                 # z in 2-m-tile tiles: the tail sigmoids wait only on their
                 # own bank's matmuls instead of all of z.
                 _ZB = [(0, 2), (2, 4), (4, 6), (6, 7), (7, 8)]
                 ps_z = [
                     gpool.tile([P, z1 - z0, BL], f32, tag="ps",
                                name=f"psz{i}_{t}")
                     for i, (z0, z1) in enumerate(_ZB)
                 ]

                 def sl(pair, m):
                     return pair[m // HM][:, m % HM, :]

                 def slz(m):
                     for i, (z0, z1) in enumerate(_ZB):
                         if z0 <= m < z1:
                             return ps_z[i][:, m - z0, :]

                 def mm_r(m):
                     out = ps_r[m // 2][:, m % 2, :]
                     for k in range(KH):
                         nc.tensor.matmul(
                             out, wrz_s[:, 1 + k, m, :], hbk(k),
                             start=(k == 0), stop=False,
                         )
                     nc.tensor.matmul(
                         out, wrz_s[:, 0, m, :], x_b[:], start=False, stop=True
                     )

                 def mm_hn(m):
                     out = ps_hn[m // 2][:, m % 2, :]
                     for k in range(KH):
                         nc.tensor.matmul(
                             out, wnh_s[:, k, m, :], hbk(k),
                             start=(k == 0), stop=(k == KH - 1),
                         )

                 # PE emission order: r/hn pairs (chain-critical first), xn
                 # early (needs only x), z last (shallow post-chain).
                 mm_r(0); mm_hn(0); mm_r(1); mm_hn(1)
                 for m in range(KH):
                     nc.tensor.matmul(
                         sl(ps_xn, m), wnx_s[:, m, :], x_b[:],
                         start=True, stop=True,
                     )
                 for m in range(2, KH):
                     mm_r(m); mm_hn(m)
                 for m in range(KH):
                     out = slz(m)
                     for k in range(KH):
                         nc.tensor.matmul(
                             out, wrz_s[:, 1 + k, KH + m, :], hbk(k),
                             start=(k == 0), stop=False,
                         )
                     nc.tensor.matmul(
                         out, wrz_s[:, 0, KH + m, :], x_b[:],
                         start=False, stop=True,
                     )

                 if skeleton:
                     continue  # timing experiment: matmul stream only

                 # Chunked per-tile pipeline: every chunk tensor is its own
                 # tile so readers wait only on their chunk's writers.
                 r_s = [
                     wpool.tile([P, 2, BL], f32, tag=f"r{i}", name=f"r{i}_{t}")
                     for i in range(4)
                 ]
                 t1 = [
                     wpool.tile([P, 2, BL], f32, tag=f"t1{i}", name=f"t1{i}_{t}")
                     for i in range(4)
                 ]
                 t2c = [
                     wpool.tile([P, c1 - c0, BL], f32, tag=f"t2{i}",
                                name=f"t2{i}_{t}")
                     for i, (c0, c1) in enumerate(_SC)
                 ]
                 n_c = [
                     wpool.tile([P, c1 - c0, BL], f32, tag=f"n{i}",
                                name=f"n{i}_{t}")
                     for i, (c0, c1) in enumerate(_SC)
                 ]
                 d_c = [
                     wpool.tile([P, c1 - c0, BL], f32, tag=f"d{i}",
                                name=f"d{i}_{t}")
                     for i, (c0, c1) in enumerate(_SC)
                 ]
                 z_c = [
                     wpool.tile([P, c1 - c0, BL], f32, tag=f"z{i}",
                                name=f"z{i}_{t}")
                     for i, (c0, c1) in enumerate(_SC)
                 ]
                 e_c = [
                     wpool.tile([P, c1 - c0, BL], f32, tag=f"e{i}",
                                name=f"e{i}_{t}")
                     for i, (c0, c1) in enumerate(_SC)
                 ]
                 hf2 = [
                     spool.tile([P, c1 - c0, BL], f32, tag=f"hf{i}",
                                name=f"hf{i}_{t}")
                     for i, (c0, c1) in enumerate(_SC)
                 ]
                 hb2 = [
                     spool.tile([P, c1 - c0, BL], bf16, tag=f"hb{i}",
                                name=f"hb{i}_{t}")
                     for i, (c0, c1) in enumerate(_SC)
                 ]

                 def t2sl(m):
                     i = _CHUNK_OF[m]
                     return t2c[i][:, m - _SC[i][0], :]

                 def zsl(m):
                     i = _CHUNK_OF[m]
                     return z_c[i][:, m - _SC[i][0], :]

                 def sig_r(m):
                     nc.scalar.activation(
                         r_s[m // 2][:, m % 2, :], ps_r[m // 2][:, m % 2, :],
                         AF.Sigmoid, bias=brz_s[:, m : m + 1],
                     )

                 def t12(m):
                     nc.vector.scalar_tensor_tensor(
                         t1[m // 2][:, m % 2, :], ps_hn[m // 2][:, m % 2, :],
                         bhn_s[:, m : m + 1], r_s[m // 2][:, m % 2, :],
                         op0=OP.add, op1=OP.mult,
                     )
                     nc.vector.scalar_tensor_tensor(
                         t2sl(m), sl(ps_xn, m), bxn_s[:, m : m + 1],
                         t1[m // 2][:, m % 2, :], op0=OP.add, op1=OP.add,
                     )

                 def tanh_chunk(i):
                     nc.scalar.activation(n_c[i][:], t2c[i][:], AF.Tanh)

                 chain = nc.gpsimd if pool_chain else nc.vector

                 def d_chunk(i):
                     chain.tensor_sub(d_c[i][:], h_f[i][:], n_c[i][:])

                 def sig_z(m):
                     nc.scalar.activation(
                         zsl(m), slz(m), AF.Sigmoid,
                         bias=brz_s[:, KH + m : KH + m + 1],
                     )

                 def ehc_chunk(i, eng=None):
                     eng = eng or chain
                     eng.tensor_mul(e_c[i][:], z_c[i][:], d_c[i][:])
                     eng.tensor_add(hf2[i][:], n_c[i][:], e_c[i][:])
                     eng.tensor_copy(hb2[i][:], hf2[i][:])

                 # Emission interleave: per-engine order matches readiness
                 sig_r(0); sig_r(1); sig_r(2); sig_r(3)
                 t12(0); t12(1); t12(2); t12(3)
                 sig_r(4); sig_r(5)
                 t12(4); t12(5)
                 tanh_chunk(0)
                 sig_r(6); sig_r(7)
                 t12(6); t12(7)
                 tanh_chunk(1)
                 for m in range(4):
                     sig_z(m)
                 tanh_chunk(2); tanh_chunk(3)
                 for m in range(4, KH):
                     sig_z(m)

                 d_chunk(0); d_chunk(1)
                 ehc_chunk(0, nc.vector)
                 d_chunk(2); d_chunk(3)
                 ehc_chunk(1); ehc_chunk(2); ehc_chunk(3)

                 # tp = [[lp_W],[fc_p@lp_W + fc_h]] @ h_n  (one matmul set)
                 ps_tp_t = tpool.tile(
                     [P, HM, BL], f32, tag="tp", name=f"pstp_{t}"
                 )
                 ps_tp = ps_tp_t[:, 0, :]
                 for k in range(KH):
                     i = _CHUNK_OF[k]
                     nc.tensor.matmul(
                         ps_tp, wtp_s[:, k, :], hb2[i][:, k - _SC[i][0], :],
                         start=(k == 0), stop=(k == KH - 1),
                     )

                 # y = x + tp + btp ; y becomes x
                 x_f2 = spool.tile([P, BL], f32, tag="xf")
                 nc.vector.scalar_tensor_tensor(
                     x_f2[:], ps_tp, btp_s[:, 0:1], x_f[:],
                     op0=OP.add, op1=OP.add,
                 )
                 x_b2 = spool.tile([P, BL], bf16, tag="xb")
                 nc.vector.tensor_copy(x_b2[:], x_f2[:])
                 nc.sync.dma_start(yt_d[t, :, :], x_f2[:])

                 x_f, x_b, h_f, h_b = x_f2, x_b2, hf2, hb2

    nc.compile()
    return nc


def _build_kmaj(
    t_steps, reps=1, skeleton=False, gps_tail=False,
    hf2_gps=False, c0_singles=True, sr_first=True,
    no_zmk=True, warm_pe=0,
):
    """k-major emission: gate matmuls grouped by h k-tile so the PE starts
    step t+1's stream as soon as chain t finishes its first h' pair, instead
    of waiting for the full chain.

    PSUM: per-accumulator tags (bufs=1) so a start-matmul of step t+1 waits
    only on the *early* chain ops of step t that read the same slot:
      r(m) slot <- sigmoid_r(t,m); hn(m) <- t1(t,m); z(m) <- sigmoid_z(t,m);
      xn pair <- t2(t,m..); tp rides xn0's slot (its next-step user already
      depends on y(t) via x).
    z accumulators start at k-group 1 (k=0 made up during the x-pass) so
    their start-matmuls arrive after sigmoid_z(t,m) has freed the slot.

    reps>1 wraps the step loop in For_i with state copy-back (timing).
    """
    import contextlib

    import concourse.bass as bass  # noqa: F401
    import concourse.tile as tile
    from concourse import bacc, mybir

    f32 = mybir.dt.float32
    bf16 = mybir.dt.bfloat16
    AF = mybir.ActivationFunctionType
    OP = mybir.AluOpType

    nc = bacc.Bacc(None, target_bir_lowering=False, debug=False)

    dp = nc.declare_dram_parameter
    x0_d = dp("x0", [P, BL], f32, isOutput=False)
    h0_d = dp("h0", [P, KH, BL], f32, isOutput=False)
    wrz_d = dp("wrz", [P, 9, 16, P], bf16, isOutput=False)
    wnx_d = dp("wnx", [P, KH, P], bf16, isOutput=False)
    wnh_d = dp("wnh", [P, KH, KH, P], bf16, isOutput=False)
    wtp_d = dp("wtp", [P, KH, P], bf16, isOutput=False)
    brz_d = dp("brz", [P, 16], f32, isOutput=False)
    bxn_d = dp("bxn", [P, KH], f32, isOutput=False)
    bhn_d = dp("bhn", [P, KH], f32, isOutput=False)
    btp_d = dp("btp", [P, 1], f32, isOutput=False)
    yt_d = dp("yt", [t_steps, P, BL], f32, isOutput=True)

    NP = KH // 2  # h-state pairs

    with tile.TileContext(nc) as tc:
        with (
            tc.tile_pool(name="const", bufs=1) as cpool,
            tc.tile_pool(name="state", bufs=2) as spool,
            tc.tile_pool(name="work", bufs=2) as wpool,
            tc.tile_pool(name="gates_ps", bufs=1, space="PSUM") as gp,
        ):
            def load_const(dram, shape, dtype):
                t = cpool.tile(shape, dtype, tag=dram.name, name=dram.name)
                nc.sync.dma_start(t[:], dram[:])
                return t

            wrz_s = load_const(wrz_d, [P, 9, 16, P], bf16)
            wnx_s = load_const(wnx_d, [P, KH, P], bf16)
            wnh_s = load_const(wnh_d, [P, KH, KH, P], bf16)
            wtp_s = load_const(wtp_d, [P, KH, P], bf16)
            brz_s = load_const(brz_d, [P, 16], f32)
            bxn_s = load_const(bxn_d, [P, KH], f32)
            bhn_s = load_const(bhn_d, [P, KH], f32)
            btp_s = load_const(btp_d, [P, 1], f32)

            # initial state: cpool (bufs=1) so reps>1 can copy back into it
            hin_f = [
                cpool.tile([P, 2, BL], f32, tag=f"hinf{i}", name=f"hinf{i}")
                for i in range(NP)
            ]
            hin_b = [
                cpool.tile([P, 2, BL], bf16, tag=f"hinb{i}", name=f"hinb{i}")
                for i in range(NP)
            ]
            for i in range(NP):
                nc.sync.dma_start(hin_f[i][:], h0_d[:, 2 * i : 2 * i + 2, :])
                nc.vector.tensor_copy(hin_b[i][:], hin_f[i][:])
            xin_f = cpool.tile([P, BL], f32, tag="xinf", name="xinf")
            nc.sync.dma_start(xin_f[:], x0_d[:])
            xin_b = cpool.tile([P, BL], bf16, tag="xinb", name="xinb")
            nc.vector.tensor_copy(xin_b[:], xin_f[:])

            T12E = nc.vector  # t1/t2 read PSUM; GpSimd has no PSUM port
            TAILE = nc.gpsimd if gps_tail else nc.vector

            rep_ctx = (
                tc.For_i(0, reps, 1) if reps > 1 else contextlib.nullcontext()
            )
            with rep_ctx:
                x_f, x_b = xin_f, xin_b
                h_f, h_b = list(hin_f), list(hin_b)

                for t in range(t_steps):
                    def hbk(k):
                        return h_b[k // 2][:, k % 2, :]

                    # PSUM: 8 banks, one [P,4,BL] tile each (bank-granular
                    # slots); 4 m-slices per bank, subtile deps keep reader
                    # granularity fine.
                    ps_r = [
                        gp.tile([P, 4, BL], f32, tag=f"r{i}", name=f"psr{i}_{t}")
                        for i in range(2)
                    ]
                    ps_hn = [
                        gp.tile([P, 4, BL], f32, tag=f"hn{i}", name=f"pshn{i}_{t}")
                        for i in range(2)
                    ]
                    ps_z = [
                        gp.tile([P, 4, BL], f32, tag=f"z{i}", name=f"psz{i}_{t}")
                        for i in range(2)
                    ]
                    ps_xn = [
                        gp.tile([P, 4, BL], f32, tag=f"xn{i}", name=f"psxn{i}_{t}")
                        for i in range(2)
                    ]

                    def rsl(m):
                        return ps_r[m // 4][:, m % 4, :]

                    def hnsl(m):
                        return ps_hn[m // 4][:, m % 4, :]

                    def zsl(m):
                        return ps_z[m // 4][:, m % 4, :]

                    def xnsl(m):
                        return ps_xn[m // 4][:, m % 4, :]

                    # ---- gate matmul stream, k-major ----------------------
                    # One accumulation group per PSUM bank (2KB zero region):
                    # start only on the first matmul touching the bank, stop
                    # only on the last; fresh slices are overwritten via
                    # per-element has_written bits.
                    for g in range(KH):
                        for m in range(KH):
                            nc.tensor.matmul(
                                rsl(m), wrz_s[:, 1 + g, m, :], hbk(g),
                                start=(g == 0 and m % 4 == 0), stop=False,
                            )
                            nc.tensor.matmul(
                                hnsl(m), wnh_s[:, g, m, :], hbk(g),
                                start=(g == 0 and m % 4 == 0),
                                stop=(g == KH - 1 and m % 4 == 3),
                            )
                        if g >= 1 or no_zmk:
                            # z starts at k-group 1 (k=0 made up below) unless
                            # no_zmk, which starts it at k-group 0 directly
                            zstart = 0 if no_zmk else 1
                            for m in range(KH):
                                nc.tensor.matmul(
                                    zsl(m), wrz_s[:, 1 + g, KH + m, :], hbk(g),
                                    start=(g == zstart and m % 4 == 0),
                                    stop=False,
                                )
                    # x-pass (+ z k=0 makeup), m-interleaved
                    for m in range(KH):
                        if not no_zmk:
                            nc.tensor.matmul(
                                zsl(m), wrz_s[:, 1, KH + m, :], hbk(0),
                                start=False, stop=False,
                            )
                        nc.tensor.matmul(
                            rsl(m), wrz_s[:, 0, m, :], x_b[:],
                            start=False, stop=(m % 4 == 3),
                        )
                        nc.tensor.matmul(
                            zsl(m), wrz_s[:, 0, KH + m, :], x_b[:],
                            start=False, stop=(m % 4 == 3),
                        )
                        nc.tensor.matmul(
                            xnsl(m), wnx_s[:, m, :], x_b[:],
                            start=(m % 4 == 0), stop=(m % 4 == 3),
                        )

                    # dummy weight loads: keep the PE array active through
                    # the chain-head window so the p-state ramp doesn't reset
                    # (no PSUM writes; real matmuls self-load their weights)
                    for w in range(warm_pe):
                        nc.tensor.ldweights(wrz_s[:, 1 + w % 8, w % 16, :])

                    if skeleton:
                        continue

                    # ---- chain ------------------------------------------
                    r_s = [
                        wpool.tile([P, 2, BL], f32, tag=f"rs{i}", name=f"rs{i}_{t}")
                        for i in range(4)
                    ]
                    z_s = [
                        wpool.tile([P, 2, BL], f32, tag=f"zs{i}", name=f"zs{i}_{t}")
                        for i in range(4)
                    ]
                    t1 = [
                        wpool.tile([P, BL], f32, tag=f"t1{m}", name=f"t1{m}_{t}")
                        for m in range(KH)
                    ]
                    t2 = [
                        wpool.tile([P, 2, BL], f32, tag=f"t2{i}", name=f"t2{i}_{t}")
                        for i in range(4)
                    ]
                    n_s = [
                        wpool.tile([P, 2, BL], f32, tag=f"n{i}", name=f"n{i}_{t}")
                        for i in range(4)
                    ]
                    d_c = [
                        wpool.tile([P, 2, BL], f32, tag=f"d{i}", name=f"d{i}_{t}")
                        for i in range(4)
                    ]
                    e_c = [
                        wpool.tile([P, 2, BL], f32, tag=f"e{i}", name=f"e{i}_{t}")
                        for i in range(4)
                    ]
                    hf2 = [
                        spool.tile([P, 2, BL], f32, tag=f"hf{i}", name=f"hf{i}_{t}")
                        for i in range(NP)
                    ]
                    hb2 = [
                        spool.tile([P, 2, BL], bf16, tag=f"hb{i}", name=f"hb{i}_{t}")
                        for i in range(NP)
                    ]
                    ps_tp = gp.tile([P, 1, BL], f32, tag="xn0", name=f"pstp_{t}")

                    def sig_r(m):
                        nc.scalar.activation(
                            r_s[m // 2][:, m % 2, :], rsl(m), AF.Sigmoid,
                            bias=brz_s[:, m : m + 1],
                        )

                    def sig_z(m):
                        nc.scalar.activation(
                            z_s[m // 2][:, m % 2, :], zsl(m), AF.Sigmoid,
                            bias=brz_s[:, KH + m : KH + m + 1],
                        )

                    def t12(m):
                        T12E.scalar_tensor_tensor(
                            t1[m][:], hnsl(m), bhn_s[:, m : m + 1],
                            r_s[m // 2][:, m % 2, :], op0=OP.add, op1=OP.mult,
                        )
                        T12E.scalar_tensor_tensor(
                            t2[m // 2][:, m % 2, :], xnsl(m),
                            bxn_s[:, m : m + 1], t1[m][:],
                            op0=OP.add, op1=OP.add,
                        )

                    def tanh_p(i):
                        nc.scalar.activation(n_s[i][:], t2[i][:], AF.Tanh)

                    HF2E = nc.gpsimd if hf2_gps else TAILE

                    def tail_p(i):
                        # h' = n + z*(h - n); bf16 form first (gates the PE),
                        # f32 state copy off the critical path
                        TAILE.tensor_sub(d_c[i][:], h_f[i][:], n_s[i][:])
                        TAILE.tensor_mul(e_c[i][:], z_s[i][:], d_c[i][:])
                        TAILE.tensor_add(hb2[i][:], n_s[i][:], e_c[i][:])
                        HF2E.tensor_add(hf2[i][:], n_s[i][:], e_c[i][:])

                    def tail_s(i, j):
                        # single-k tail (lower latency for chunk 0)
                        sl = (slice(None), slice(j, j + 1), slice(None))
                        nc.scalar.activation(n_s[i][sl], t2[i][sl], AF.Tanh)
                        TAILE.tensor_sub(d_c[i][sl], h_f[i][sl], n_s[i][sl])
                        TAILE.tensor_mul(e_c[i][sl], z_s[i][sl], d_c[i][sl])
                        TAILE.tensor_add(hb2[i][sl], n_s[i][sl], e_c[i][sl])
                        HF2E.tensor_add(hf2[i][sl], n_s[i][sl], e_c[i][sl])

                    def tp_mm(k):
                        nc.tensor.matmul(
                            ps_tp[:, 0, :], wtp_s[:, k, :],
                            hb2[k // 2][:, k % 2, :],
                            start=(k == 0), stop=(k == KH - 1),
                        )

                    def emit_pair(i):
                        if c0_singles and i == 0:
                            tail_s(0, 0)
                            tp_mm(0)
                            tail_s(0, 1)
                            tp_mm(1)
                            return
                        tanh_p(i)
                        tail_p(i)
                        tp_mm(2 * i)
                        tp_mm(2 * i + 1)

                    if sr_first:
                        # ACT order: all sigmoid_r first (critical: t1 needs
                        # r), sigmoid_z just before its consumers
                        for m in range(KH):
                            sig_r(m)
                            t12(m)
                            if m >= 2 and m % 2 == 0:
                                i = m // 2 - 1
                                sig_z(2 * i)
                                sig_z(2 * i + 1)
                                emit_pair(i)
                        sig_z(6)
                        sig_z(7)
                        emit_pair(3)
                    else:
                        for m in range(KH):
                            sig_r(m)
                            sig_z(m)
                            t12(m)
                            if m >= 2 and m % 2 == 0:
                                emit_pair(m // 2 - 1)
                        emit_pair(3)

                    # y = x + tp + btp ; y becomes x (bf16 first: gates x-pass)
                    x_b2 = spool.tile([P, BL], bf16, tag="xb", name=f"xb_{t}")
                    nc.vector.scalar_tensor_tensor(
                        x_b2[:], ps_tp[:, 0, :], btp_s[:, 0:1], x_f[:],
                        op0=OP.add, op1=OP.add,
                    )
                    x_f2 = spool.tile([P, BL], f32, tag="xf", name=f"xf_{t}")
                    nc.vector.scalar_tensor_tensor(
                        x_f2[:], ps_tp[:, 0, :], btp_s[:, 0:1], x_f[:],
                        op0=OP.add, op1=OP.add,
                    )
                    nc.sync.dma_start(yt_d[t, :, :], x_f2[:])

                    x_f, x_b, h_f, h_b = x_f2, x_b2, hf2, hb2

                if reps > 1 and not skeleton:
                    for i in range(NP):
                        nc.vector.tensor_copy(hin_f[i][:], h_f[i][:])
                        nc.vector.tensor_copy(hin_b[i][:], h_b[i][:])
                    nc.vector.tensor_copy(xin_f[:], x_f[:])
                    nc.vector.tensor_copy(xin_b[:], x_b[:])

    nc.compile()
    return nc


def _build(t_steps, reps=1, skeleton=False, order=None, **kw):
    if order is None:
        order = os.environ.get("KERNEL_ORDER", "kmaj")
    if order == "kmaj":
        return _build_kmaj(t_steps, reps=reps, skeleton=skeleton, **kw)
    return _build_mmaj(t_steps, reps=reps, skeleton=skeleton, **kw)


def _prep_inputs(h, gt, Wih, Whh, bih, bhh, lp_W, lp_b, fc_W, fc_b):
    """Host-side: transpose into kernel layouts, cast weights to bf16."""
    bf = ml_dtypes.bfloat16
    f32 = np.float32

    # rz combined weights, transposed: [1152, 2048] -> [p, k(9), m(16), j]
    wrzT = np.concatenate([Wih[: 2 * H].T, Whh[: 2 * H].T], axis=0)
    wrz = np.empty((P, 9, 16, P), dtype=bf)
    for k in range(9):
        for m in range(16):
            wrz[:, k, m, :] = wrzT[k * P : (k + 1) * P, m * P : (m + 1) * P]

    wnxT = Wih[2 * H :].T  # [128, 1024]
    wnx = np.ascontiguousarray(wnxT.reshape(P, KH, P), dtype=bf)  # [p, m, j]

    wnhT = Whh[2 * H :].T  # [1024, 1024]
    wnh = np.empty((P, KH, KH, P), dtype=bf)
    for k in range(KH):
        for m in range(KH):
            wnh[:, k, m, :] = wnhT[k * P : (k + 1) * P, m * P : (m + 1) * P]

    # fold pose->traj head: traj = (fc_p@lp_W + fc_h)@h + (fc_p@lp_b + fc_b)
    fc_p = fc_W[:, :POSE].astype(np.float64)
    fc_h = fc_W[:, POSE:].astype(np.float64)
    m_traj = fc_p @ lp_W.astype(np.float64) + fc_h          # [32, 1024]
    m_tp = np.concatenate([m_traj, lp_W.astype(np.float64)], axis=0)  # [I, H]
    b_traj = fc_p @ lp_b.astype(np.float64) + fc_b          # [32]
    b_tp = np.concatenate([b_traj, lp_b.astype(np.float64)])  # [I]
    wtpT = m_tp.T  # [1024, 128]
    wtp = np.ascontiguousarray(
        wtpT.reshape(KH, P, P).transpose(1, 0, 2), dtype=bf
    )  # [p, k, m]

    b_rz = (bih + bhh)[: 2 * H].astype(f32)  # [2048]
    brz = np.ascontiguousarray(b_rz.reshape(16, P).T)  # [128, 16]
    bxn = np.ascontiguousarray(bih[2 * H :].reshape(KH, P).T.astype(f32))
    bhn = np.ascontiguousarray(bhh[2 * H :].reshape(KH, P).T.astype(f32))
    btp = b_tp.reshape(P, 1).astype(f32)

    shared = {
        "wrz": wrz, "wnx": wnx, "wnh": wnh, "wtp": wtp,
        "brz": brz, "bxn": bxn, "bhn": bhn, "btp": btp,
    }

    in_maps = []
    for c in range(NCORES):
        sl = slice(c * BL, (c + 1) * BL)
        x0 = np.ascontiguousarray(gt[sl, 0, :].T.astype(f32))  # [I, BL]
        h0 = np.ascontiguousarray(
            h[sl, :].T.reshape(KH, P, BL).transpose(1, 0, 2).astype(f32)
        )  # [p, k, b] = h[b, k*128+p]
        in_maps.append({"x0": x0, "h0": h0, **shared})
    return in_maps


def kernel(h, gt, Wih, Whh, bih, bhh, lp_W, lp_b, fc_W, fc_b, time_steps):
    from concourse.bass_utils import run_bass_kernel_spmd

    t_steps = int(time_steps)

    h = np.asarray(h, np.float32)
    gt = np.asarray(gt, np.float32)

    if t_steps not in _BUILD_CACHE:
        _BUILD_CACHE[t_steps] = _build(t_steps)
    nc = _BUILD_CACHE[t_steps]

    in_maps = _prep_inputs(
        h, gt, np.asarray(Wih, np.float32), np.asarray(Whh, np.float32),
        np.asarray(bih, np.float32), np.asarray(bhh, np.float32),
        np.asarray(lp_W, np.float32), np.asarray(lp_b, np.float32),
        np.asarray(fc_W, np.float32), np.asarray(fc_b, np.float32),
    )

    import os

    trace = bool(os.environ.get("KERNEL_TRACE"))
    res = run_bass_kernel_spmd(
        nc, in_maps, core_ids=list(range(NCORES)), trace=trace
    )
    global LAST_RESULTS
    LAST_RESULTS = res

    out = np.empty((B, t_steps, I), dtype=np.float32)
    for c in range(NCORES):
        yt = res.results[c]["yt"]  # [T, I_k, BL]
        out[c * BL : (c + 1) * BL] = yt.transpose(2, 0, 1)
    return out

